# revision 21
# baseline (speedup 1.0000x reference)
"""DAE loss kernel for Trainium2 (Bass/Tile), 8-core data parallel.

Shards the batch (512 -> 64 rows/core). Each core computes, per (b,s)
position: argmax over V (scan-cummax + count trick), x[target] (one-hot
scalar_tensor_tensor gather), log-sum-exp (ACT Exp with accumulate), and
a PE-matmul contraction for the label-smoothing mean term. A small [B,S]
stage reduces everything to 10 per-row partial sums; the host sums the
8x64 partial rows and applies the final scalar formulas.
"""

import numpy as np
from contextlib import ExitStack

import concourse.bass as bass
import concourse.bacc as bacc
import concourse.mybir as mybir
from concourse import tile
from concourse.bass_utils import run_bass_kernel_spmd


B, S, V = 512, 128, 1024
NCORES = 8
BL = B // NCORES  # 64 batch rows per core

PAD_IDX = 0
LABEL_SMOOTHING = 0.1
END_WEIGHT = 3.0
CHAR_WEIGHT = 0.2
LENGTH_PENALTY = 0.1

F32 = mybir.dt.float32
F16 = mybir.dt.float16
I32 = mybir.dt.int32
OP = mybir.AluOpType
AF = mybir.ActivationFunctionType
AX = mybir.AxisListType

NEG_INF = -3.0e38

# gpsimd cannot run scalar_tensor_tensor (walrus opcode-on-engine check),
# so the one-hot gather lives on the vector engine.
GATHER_ON_GPSIMD = 0


def build_bass():
    # Bacc (not raw Bass): its compile() splits multi-sem waits into
    # EventSemaphore chains — hardware allows one sync wait per instruction.
    nc = bacc.Bacc("TRN2", target_bir_lowering=False, debug=False, num_devices=NCORES)
    x_d = nc.dram_tensor("output", [BL, S, V], F32, kind="ExternalInput").ap()
    t_d = nc.dram_tensor("target", [BL, S], I32, kind="ExternalInput").ap()
    stats_d = nc.dram_tensor("stats", [BL, 16], F32, kind="ExternalOutput").ap()
    aux_d = nc.dram_tensor("aux", [1, 4], F32, kind="ExternalOutput").ap()

    with tile.TileContext(nc) as tc, ExitStack() as ctx:
        const = ctx.enter_context(tc.tile_pool(name="const", bufs=1))
        cols = ctx.enter_context(tc.tile_pool(name="cols", bufs=1))
        small = ctx.enter_context(tc.tile_pool(name="small", bufs=1))
        xpool = ctx.enter_context(tc.tile_pool(name="x", bufs=4))
        cmpool = ctx.enter_context(tc.tile_pool(name="cm", bufs=3))
        dpool = ctx.enter_context(tc.tile_pool(name="dump", bufs=6))
        psum = ctx.enter_context(tc.tile_pool(name="psum", bufs=2, space="PSUM"))
        psacc = ctx.enter_context(tc.tile_pool(name="psacc", bufs=1, space="PSUM"))

        # ---- constants ----
        iota_v_i = const.tile([128, V], I32, tag="iotavi")
        nc.gpsimd.iota(iota_v_i[:], [[1, V]], base=0, channel_multiplier=0)
        iota_v = const.tile([128, V], F32, tag="iotav")
        nc.vector.tensor_copy(iota_v[:], iota_v_i[:])

        # identities produced by DVE (not gpsimd) so each PE transpose
        # depends on a single engine: walrus allows ONE sync wait per Matmult.
        pcol_i = const.tile([128, 1], I32, tag="pcoli")
        nc.gpsimd.iota(pcol_i[:], [[0, 1]], base=0, channel_multiplier=1)
        pcolf = const.tile([128, 1], F32, tag="pcolf")
        nc.vector.tensor_copy(pcolf[:], pcol_i[:])
        ident128 = const.tile([128, 128], F32, tag="id128")
        nc.vector.tensor_scalar(
            ident128[:], iota_v[:, 0:128], pcolf[:], None, OP.is_equal
        )
        ident64 = const.tile([64, 64], F32, tag="id64")
        nc.vector.tensor_scalar(
            ident64[:], iota_v[0:64, 0:64], pcolf[0:64], None, OP.is_equal
        )

        iota_s_i = const.tile([BL, S], I32, tag="iotasi")
        nc.gpsimd.iota(iota_s_i[:], [[1, S]], base=0, channel_multiplier=0)
        iota_s = const.tile([BL, S], F32, tag="iotas")
        nc.vector.tensor_copy(iota_s[:], iota_s_i[:])

        # ---- stage 1: target-derived weights ----
        tgt_i = small.tile([BL, S], I32, tag="tgti")
        nc.sync.dma_start(tgt_i[:], t_d[:, :])
        tgt_f = small.tile([BL, S], F32, tag="tgtf")
        nc.vector.tensor_copy(tgt_f[:], tgt_i[:])

        mask = small.tile([BL, S], F32, tag="mask")
        nc.vector.tensor_scalar(mask[:], tgt_f[:], float(PAD_IDX), None, OP.not_equal)

        L = small.tile([BL, 1], F32, tag="L")
        nc.vector.tensor_reduce(L[:], mask[:], AX.X, OP.add)
        Lf = small.tile([BL, 1], F32, tag="Lf")
        nc.vector.tensor_scalar(Lf[:], L[:], 1.0, None, OP.max)
        rec = small.tile([BL, 1], F32, tag="rec")
        nc.vector.reciprocal(rec[:], Lf[:])
        halfinv = small.tile([BL, 1], F32, tag="halfinv")
        nc.vector.tensor_scalar(halfinv[:], rec[:], 0.5, None, OP.mult)

        base = small.tile([BL, S], F32, tag="base")
        nc.vector.tensor_scalar(base[:], iota_s[:], halfinv[:], None, OP.mult)
        cmplt = small.tile([BL, S], F32, tag="cmplt")
        nc.vector.tensor_scalar(cmplt[:], iota_s[:], L[:], None, OP.is_lt)
        w = small.tile([BL, S], F32, tag="w")
        nc.vector.tensor_tensor(w[:], base[:], cmplt[:], OP.mult)
        nc.vector.tensor_scalar(w[:], w[:], 1.0, None, OP.add)

        # end-of-sequence boosts; positions L-3, L-2, L-1 are distinct so the
        # replacements commute (negative L-k never matches iota_s >= 0).
        for k, c in ((3, END_WEIGHT * 0.6), (2, END_WEIGHT * 0.8), (1, END_WEIGHT)):
            Lk = small.tile([BL, 1], F32, tag=f"Lk{k}")
            nc.vector.tensor_scalar(Lk[:], L[:], float(k), None, OP.subtract)
            eqk = small.tile([BL, S], F32, tag=f"eqk{k}")
            nc.vector.tensor_scalar(eqk[:], iota_s[:], Lk[:], None, OP.is_equal)
            d = small.tile([BL, S], F32, tag=f"d{k}")
            nc.vector.tensor_scalar(d[:], w[:], -1.0, float(c), OP.mult, OP.add)
            nc.vector.tensor_tensor(d[:], d[:], eqk[:], OP.mult)
            nc.vector.tensor_tensor(w[:], w[:], d[:], OP.add)

        wm = small.tile([BL, S], F32, tag="wm")
        nc.vector.tensor_tensor(wm[:], w[:], mask[:], OP.mult)

        # transposes to [128(s), 64(b)] for per-tile per-partition scalars
        wmT_ps = psum.tile([S, BL], F32, tag="tps")
        nc.tensor.transpose(wmT_ps[:], wm[:], ident64[:])
        wmT = cols.tile([S, BL], F32, tag="wmT")
        nc.scalar.copy(wmT[:], wmT_ps[:])

        tT_ps = psum.tile([S, BL], F32, tag="tps")
        nc.tensor.transpose(tT_ps[:], tgt_f[:], ident64[:])
        tT = cols.tile([S, BL], F32, tag="tT")
        nc.scalar.copy(tT[:], tT_ps[:])

        # gpsimd allows only one sync wait per instruction, and its gathers
        # must wait on their x-tile DMA. Absorb the iota_v/tT producer waits
        # into tiny same-engine probe copies ahead of the loop so the gathers'
        # vector clock is already up to date for those operands.
        probe = const.tile([1, 2], F32, tag="probe")
        nc.gpsimd.tensor_copy(probe[:, 0:1], iota_v[0:1, 0:1])
        nc.gpsimd.tensor_copy(probe[:, 1:2], tT[0:1, 0:1])

        # ---- stage 2: the V loop ----
        idxT = cols.tile([S, BL], F32, tag="idxT")
        xtT = cols.tile([S, BL], F32, tag="xtT")
        seT = cols.tile([S, BL], F32, tag="seT")

        sx0 = psacc.tile([1, 512], F32, tag="sx0")
        sx1 = psacc.tile([1, 512], F32, tag="sx1")

        # dummy matmul reading only wmT: absorbs the ACT(copy)->PE dependency
        # so the j=0 matmul below carries a single DMA wait (walrus limit).
        dummy_ps = psacc.tile([1, 1], F32, tag="dummy")
        nc.tensor.matmul(dummy_ps[:], wmT[:, 0:1], wmT[:, 0:1])

        for j in range(BL):
            xj = xpool.tile([S, V], F32, tag="xj")
            nc.sync.dma_start(xj[:], x_d[j])

            # running max along V; last column is the row max
            cm = cmpool.tile([S, V], F32, tag="cm")
            nc.vector.tensor_tensor_scan(
                cm[:], xj[:], xj[:], NEG_INF, OP.max, OP.bypass
            )
            # argmax = #positions strictly before the first max
            dcnt = dpool.tile([S, V], F16, tag="dumpc")
            nc.vector.tensor_scalar(
                dcnt[:], cm[:], cm[:, V - 1 : V], None, OP.is_lt, OP.add,
                accum_out=idxT[:, j : j + 1],
            )
            # sum of exp(x) along V (no max subtraction: |x|<6 is safe in fp32)
            de = dpool.tile([S, V], F32, tag="dumpe")
            nc.scalar.activation(
                de[:], xj[:], AF.Exp, accum_out=seT[:, j : j + 1]
            )
            # gather exp(x)[target] = one-hot(iota==t) * e, sum along V; the
            # [B,S] stage recovers x_t = ln(e_t). Reading e instead of x keeps
            # gpsimd off the x tile, so the x DMA has fewer WAR waits and the
            # gpsimd gather waits only on ACT.
            dg = dpool.tile([S, V], F16, tag="dumpg")
            on_gps = j < GATHER_ON_GPSIMD
            eng = nc.gpsimd if on_gps else nc.vector
            eng.scalar_tensor_tensor(
                dg[:], iota_v[:], tT[:, j : j + 1], de[:],
                OP.is_equal, OP.mult,
                accum_out=xtT[:, j : j + 1],
            )
            # sum_v x weighted by wm, accumulated over all rows in PSUM
            nc.tensor.matmul(
                sx0[:], wmT[:, j : j + 1], xj[:, 0:512],
                start=(j == 0), stop=(j == BL - 1),
            )
            nc.tensor.matmul(
                sx1[:], wmT[:, j : j + 1], xj[:, 512:V],
                start=(j == 0), stop=(j == BL - 1),
            )

        # ---- stage 3: [B,S] wrap-up ----
        lseT = cols.tile([S, BL], F32, tag="lseT")
        nc.scalar.activation(lseT[:], seT[:], AF.Ln)

        def transpose_back(src, tag):
            ps = psum.tile([BL, S], F32, tag="tpb")
            nc.tensor.transpose(ps[:], src[:], ident128[:])
            out = small.tile([BL, S], F32, tag=tag)
            nc.scalar.copy(out[:], ps[:])
            return out

        pred = transpose_back(idxT, "pred")
        # xtT columns are written by two engines (DVE + gpsimd); merge the
        # dependency through one DVE copy so downstream ops have single waits.
        xtT2 = cols.tile([S, BL], F32, tag="xtT2")
        nc.vector.tensor_copy(xtT2[:], xtT[:])
        # xtT holds e_t = exp(x_t); recover x_t
        xtlogT = cols.tile([S, BL], F32, tag="xtlogT")
        nc.scalar.activation(xtlogT[:], xtT2[:], AF.Ln)
        xt = transpose_back(xtlogT, "xt")
        lse = transpose_back(lseT, "lse")

        stats = small.tile([BL, 16], F32, tag="stats")
        nc.vector.memset(stats[:], 0.0)
        aux = small.tile([1, 4], F32, tag="aux")
        nc.vector.memset(aux[:], 0.0)

        dump_s = small.tile([BL, S], F32, tag="dumps")

        # c0: sum_s wm * (lse - 0.9*xt)
        ce1 = small.tile([BL, S], F32, tag="ce1")
        nc.vector.scalar_tensor_tensor(
            ce1[:], xt[:], -(1.0 - LABEL_SMOOTHING), lse[:], OP.mult, OP.add
        )
        nc.vector.tensor_tensor(dump_s[:], ce1[:], wm[:], OP.mult)
        nc.vector.tensor_reduce(stats[:, 0:1], dump_s[:], AX.X, OP.add)
        # c1: sum_s w
        nc.vector.tensor_reduce(stats[:, 1:2], w[:], AX.X, OP.add)
        # c2: |pred_len - L|
        prednz = small.tile([BL, S], F32, tag="prednz")
        nc.vector.tensor_scalar(prednz[:], pred[:], float(PAD_IDX), None, OP.not_equal)
        plen = small.tile([BL, 1], F32, tag="plen")
        nc.vector.tensor_reduce(plen[:], prednz[:], AX.X, OP.add)
        pdiff = small.tile([BL, 1], F32, tag="pdiff")
        nc.vector.tensor_tensor(pdiff[:], plen[:], L[:], OP.subtract)
        nc.scalar.activation(stats[:, 2:3], pdiff[:], AF.Abs)
        # c3/c4: char bigram/trigram squared sums
        pe = small.tile([BL, S - 1], F32, tag="pe")
        nc.vector.tensor_tensor(pe[:], pred[:, 0 : S - 1], pred[:, 1:S], OP.is_equal)
        te = small.tile([BL, S - 1], F32, tag="te")
        nc.vector.tensor_tensor(te[:], tgt_f[:, 0 : S - 1], tgt_f[:, 1:S], OP.is_equal)
        same = small.tile([BL, S - 1], F32, tag="same")
        nc.vector.tensor_tensor(
            same[:], pred[:, 0 : S - 1], tgt_f[:, 0 : S - 1], OP.is_equal
        )
        pt = small.tile([BL, S - 1], F32, tag="pt")
        nc.vector.tensor_tensor(pt[:], pe[:], te[:], OP.mult)
        spt = small.tile([BL, S - 1], F32, tag="spt")
        nc.vector.tensor_tensor(spt[:], pt[:], same[:], OP.mult)
        s1 = small.tile([BL, S - 1], F32, tag="s1")
        nc.vector.tensor_tensor(s1[:], pe[:], te[:], OP.add)
        bi = small.tile([BL, S - 1], F32, tag="bi")
        nc.vector.scalar_tensor_tensor(bi[:], spt[:], -2.0, s1[:], OP.mult, OP.add)
        nc.vector.tensor_reduce(stats[:, 3:4], bi[:], AX.X, OP.add)

        pe3 = small.tile([BL, S - 2], F32, tag="pe3")
        nc.vector.tensor_tensor(pe3[:], pe[:, 0 : S - 2], pe[:, 1 : S - 1], OP.mult)
        te3 = small.tile([BL, S - 2], F32, tag="te3")
        nc.vector.tensor_tensor(te3[:], te[:, 0 : S - 2], te[:, 1 : S - 1], OP.mult)
        pt3 = small.tile([BL, S - 2], F32, tag="pt3")
        nc.vector.tensor_tensor(pt3[:], pe3[:], te3[:], OP.mult)
        spt3 = small.tile([BL, S - 2], F32, tag="spt3")
        nc.vector.tensor_tensor(spt3[:], pt3[:], same[:, 0 : S - 2], OP.mult)
        s3 = small.tile([BL, S - 2], F32, tag="s3")
        nc.vector.tensor_tensor(s3[:], pe3[:], te3[:], OP.add)
        tri = small.tile([BL, S - 2], F32, tag="tri")
        nc.vector.scalar_tensor_tensor(tri[:], spt3[:], -2.0, s3[:], OP.mult, OP.add)
        nc.vector.tensor_reduce(stats[:, 4:5], tri[:], AX.X, OP.add)
        # c5: sum_s mask[:, :-2] (valid_tri partials)
        nc.vector.tensor_reduce(stats[:, 5:6], mask[:, 0 : S - 2], AX.X, OP.add)
        # c6: correct = (pred == target) & mask
        eqc = small.tile([BL, S], F32, tag="eqc")
        nc.vector.tensor_tensor(eqc[:], pred[:], tgt_f[:], OP.is_equal)
        dump_s2 = small.tile([BL, S], F32, tag="dumps2")
        nc.vector.tensor_tensor(dump_s2[:], eqc[:], mask[:], OP.mult)
        nc.vector.tensor_reduce(stats[:, 6:7], dump_s2[:], AX.X, OP.add)
        # c7: total chars per row (= L)
        nc.vector.tensor_copy(stats[:, 7:8], L[:])
        # c8: end char ok
        Lm1 = small.tile([BL, 1], F32, tag="Lm1")
        nc.vector.tensor_scalar(Lm1[:], L[:], 1.0, None, OP.subtract)
        eqL = small.tile([BL, S], F32, tag="eqL")
        nc.vector.tensor_scalar(eqL[:], iota_s[:], Lm1[:], None, OP.is_equal)
        dump_s3 = small.tile([BL, S], F32, tag="dumps3")
        nc.vector.tensor_tensor(dump_s3[:], eqL[:], eqc[:], OP.mult)
        nc.vector.tensor_reduce(stats[:, 8:9], dump_s3[:], AX.X, OP.add)
        # c9: length accuracy partials
        nc.vector.tensor_tensor(stats[:, 9:10], plen[:], L[:], OP.is_equal)

        # aux: the two PSUM halves of sum_pos wm * sum_v x
        nc.vector.tensor_reduce(aux[:, 0:1], sx0[:], AX.X, OP.add)
        nc.vector.tensor_reduce(aux[:, 1:2], sx1[:], AX.X, OP.add)

        nc.sync.dma_start(stats_d[:, :], stats[:])
        nc.sync.dma_start(aux_d[:, :], aux[:])

    nc.compile()
    return nc


_built = None


def _get_nc():
    global _built
    if _built is None:
        _built = build_bass()
    return _built


def combine(stats_list, aux_list):
    """Host-side psum of the per-core scalar partials + final formulas."""
    Ssum = np.zeros(16, dtype=np.float64)
    Asum = np.zeros(4, dtype=np.float64)
    for st in stats_list:
        Ssum += st.astype(np.float64).sum(axis=0)
    for ax in aux_list:
        Asum += ax.astype(np.float64).sum(axis=0)

    num = Ssum[0] - (LABEL_SMOOTHING / V) * (Asum[0] + Asum[1])
    den = Ssum[1]
    weighted_loss = num / den
    length_penalty = LENGTH_PENALTY * Ssum[2] / B
    bigram_mse = Ssum[3] / (B * (S - 1) * V)
    tri_mse = Ssum[4] / (B * (S - 2) * V)
    valid_tri = Ssum[5] > 0
    char_ngram = bigram_mse + (tri_mse if valid_tri else 0.0)
    total_loss = weighted_loss + length_penalty + CHAR_WEIGHT * char_ngram

    total_chars = Ssum[7]
    char_acc = Ssum[6] / total_chars if total_chars > 0 else 0.0
    end_char_acc = Ssum[8] / B
    length_acc = Ssum[9] / B
    f = np.float32
    return (f(total_loss), f(char_acc), f(end_char_acc), f(length_acc))


def kernel(output, target, _trace=False):
    output = np.ascontiguousarray(np.asarray(output, dtype=np.float32))
    target = np.ascontiguousarray(np.asarray(target, dtype=np.int32))
    nc = _get_nc()
    in_maps = [
        {
            "output": output[c * BL : (c + 1) * BL],
            "target": target[c * BL : (c + 1) * BL],
        }
        for c in range(NCORES)
    ]
    res = run_bass_kernel_spmd(nc, in_maps, list(range(NCORES)), trace=_trace)
    stats_list = [res.results[c]["stats"] for c in range(NCORES)]
    aux_list = [res.results[c]["aux"] for c in range(NCORES)]
    out = combine(stats_list, aux_list)
    if _trace:
        return out, res
    return out


# revision 23
# speedup vs baseline: 24.8095x; 24.8095x over previous
"""DAE loss kernel for Trainium2 (Bass/Tile), 8-core data parallel.

Shards the batch (512 -> 64 rows/core). Each core computes, per (b,s)
position: argmax over V (scan-cummax + count trick), x[target] (one-hot
scalar_tensor_tensor gather), log-sum-exp (ACT Exp with accumulate), and
a PE-matmul contraction for the label-smoothing mean term. A small [B,S]
stage reduces everything to 10 per-row partial sums; the host sums the
8x64 partial rows and applies the final scalar formulas.
"""

import numpy as np
from contextlib import ExitStack

import concourse.bass as bass
import concourse.bacc as bacc
import concourse.mybir as mybir
from concourse import tile
from concourse.bass_utils import run_bass_kernel_spmd


B, S, V = 512, 128, 1024
NCORES = 8
BL = B // NCORES  # 64 batch rows per core

PAD_IDX = 0
LABEL_SMOOTHING = 0.1
END_WEIGHT = 3.0
CHAR_WEIGHT = 0.2
LENGTH_PENALTY = 0.1

F32 = mybir.dt.float32
F16 = mybir.dt.float16
I32 = mybir.dt.int32
OP = mybir.AluOpType
AF = mybir.ActivationFunctionType
AX = mybir.AxisListType

NEG_INF = -3.0e38

# gpsimd cannot run scalar_tensor_tensor (walrus opcode-on-engine check),
# so the one-hot gather lives on the vector engine.
GATHER_ON_GPSIMD = 0


def build_bass(loop_mult=1):
    # loop_mult > 1 repeats the V-loop over the same data; used only by the
    # timing bench to separate device time from dispatch overhead.
    # Bacc (not raw Bass): its compile() splits multi-sem waits into
    # EventSemaphore chains — hardware allows one sync wait per instruction.
    nc = bacc.Bacc("TRN2", target_bir_lowering=False, debug=False, num_devices=NCORES)
    x_d = nc.dram_tensor("output", [BL, S, V], F32, kind="ExternalInput").ap()
    t_d = nc.dram_tensor("target", [BL, S], I32, kind="ExternalInput").ap()
    stats_d = nc.dram_tensor("stats", [BL, 16], F32, kind="ExternalOutput").ap()
    aux_d = nc.dram_tensor("aux", [1, 4], F32, kind="ExternalOutput").ap()

    with tile.TileContext(nc) as tc, ExitStack() as ctx:
        const = ctx.enter_context(tc.tile_pool(name="const", bufs=1))
        cols = ctx.enter_context(tc.tile_pool(name="cols", bufs=1))
        small = ctx.enter_context(tc.tile_pool(name="small", bufs=1))
        xpool = ctx.enter_context(tc.tile_pool(name="x", bufs=4))
        cmpool = ctx.enter_context(tc.tile_pool(name="cm", bufs=3))
        dpool = ctx.enter_context(tc.tile_pool(name="dump", bufs=6))
        psum = ctx.enter_context(tc.tile_pool(name="psum", bufs=2, space="PSUM"))
        psacc = ctx.enter_context(tc.tile_pool(name="psacc", bufs=1, space="PSUM"))

        # ---- constants ----
        iota_v_i = const.tile([128, V], I32, tag="iotavi")
        nc.gpsimd.iota(iota_v_i[:], [[1, V]], base=0, channel_multiplier=0)
        iota_v = const.tile([128, V], F32, tag="iotav")
        nc.vector.tensor_copy(iota_v[:], iota_v_i[:])

        # identities produced by DVE (not gpsimd) so each PE transpose
        # depends on a single engine: walrus allows ONE sync wait per Matmult.
        pcol_i = const.tile([128, 1], I32, tag="pcoli")
        nc.gpsimd.iota(pcol_i[:], [[0, 1]], base=0, channel_multiplier=1)
        pcolf = const.tile([128, 1], F32, tag="pcolf")
        nc.vector.tensor_copy(pcolf[:], pcol_i[:])
        ident128 = const.tile([128, 128], F32, tag="id128")
        nc.vector.tensor_scalar(
            ident128[:], iota_v[:, 0:128], pcolf[:], None, OP.is_equal
        )
        ident64 = const.tile([64, 64], F32, tag="id64")
        nc.vector.tensor_scalar(
            ident64[:], iota_v[0:64, 0:64], pcolf[0:64], None, OP.is_equal
        )

        iota_s_i = const.tile([BL, S], I32, tag="iotasi")
        nc.gpsimd.iota(iota_s_i[:], [[1, S]], base=0, channel_multiplier=0)
        iota_s = const.tile([BL, S], F32, tag="iotas")
        nc.vector.tensor_copy(iota_s[:], iota_s_i[:])

        # ---- stage 1: target-derived weights ----
        tgt_i = small.tile([BL, S], I32, tag="tgti")
        nc.sync.dma_start(tgt_i[:], t_d[:, :])
        tgt_f = small.tile([BL, S], F32, tag="tgtf")
        nc.vector.tensor_copy(tgt_f[:], tgt_i[:])

        mask = small.tile([BL, S], F32, tag="mask")
        nc.vector.tensor_scalar(mask[:], tgt_f[:], float(PAD_IDX), None, OP.not_equal)

        L = small.tile([BL, 1], F32, tag="L")
        nc.vector.tensor_reduce(L[:], mask[:], AX.X, OP.add)
        Lf = small.tile([BL, 1], F32, tag="Lf")
        nc.vector.tensor_scalar(Lf[:], L[:], 1.0, None, OP.max)
        rec = small.tile([BL, 1], F32, tag="rec")
        nc.vector.reciprocal(rec[:], Lf[:])
        halfinv = small.tile([BL, 1], F32, tag="halfinv")
        nc.vector.tensor_scalar(halfinv[:], rec[:], 0.5, None, OP.mult)

        base = small.tile([BL, S], F32, tag="base")
        nc.vector.tensor_scalar(base[:], iota_s[:], halfinv[:], None, OP.mult)
        cmplt = small.tile([BL, S], F32, tag="cmplt")
        nc.vector.tensor_scalar(cmplt[:], iota_s[:], L[:], None, OP.is_lt)
        w = small.tile([BL, S], F32, tag="w")
        nc.vector.tensor_tensor(w[:], base[:], cmplt[:], OP.mult)
        nc.vector.tensor_scalar(w[:], w[:], 1.0, None, OP.add)

        # end-of-sequence boosts; positions L-3, L-2, L-1 are distinct so the
        # replacements commute (negative L-k never matches iota_s >= 0).
        for k, c in ((3, END_WEIGHT * 0.6), (2, END_WEIGHT * 0.8), (1, END_WEIGHT)):
            Lk = small.tile([BL, 1], F32, tag=f"Lk{k}")
            nc.vector.tensor_scalar(Lk[:], L[:], float(k), None, OP.subtract)
            eqk = small.tile([BL, S], F32, tag=f"eqk{k}")
            nc.vector.tensor_scalar(eqk[:], iota_s[:], Lk[:], None, OP.is_equal)
            d = small.tile([BL, S], F32, tag=f"d{k}")
            nc.vector.tensor_scalar(d[:], w[:], -1.0, float(c), OP.mult, OP.add)
            nc.vector.tensor_tensor(d[:], d[:], eqk[:], OP.mult)
            nc.vector.tensor_tensor(w[:], w[:], d[:], OP.add)

        wm = small.tile([BL, S], F32, tag="wm")
        nc.vector.tensor_tensor(wm[:], w[:], mask[:], OP.mult)

        # transposes to [128(s), 64(b)] for per-tile per-partition scalars
        wmT_ps = psum.tile([S, BL], F32, tag="tps")
        nc.tensor.transpose(wmT_ps[:], wm[:], ident64[:])
        wmT = cols.tile([S, BL], F32, tag="wmT")
        nc.scalar.copy(wmT[:], wmT_ps[:])

        tT_ps = psum.tile([S, BL], F32, tag="tps")
        nc.tensor.transpose(tT_ps[:], tgt_f[:], ident64[:])
        tT = cols.tile([S, BL], F32, tag="tT")
        nc.scalar.copy(tT[:], tT_ps[:])

        # gpsimd allows only one sync wait per instruction, and its gathers
        # must wait on their x-tile DMA. Absorb the iota_v/tT producer waits
        # into tiny same-engine probe copies ahead of the loop so the gathers'
        # vector clock is already up to date for those operands.
        probe = const.tile([1, 2], F32, tag="probe")
        nc.gpsimd.tensor_copy(probe[:, 0:1], iota_v[0:1, 0:1])
        nc.gpsimd.tensor_copy(probe[:, 1:2], tT[0:1, 0:1])

        # ---- stage 2: the V loop ----
        idxT = cols.tile([S, BL], F32, tag="idxT")
        xtT = cols.tile([S, BL], F32, tag="xtT")
        seT = cols.tile([S, BL], F32, tag="seT")

        sx0 = psacc.tile([1, 512], F32, tag="sx0")
        sx1 = psacc.tile([1, 512], F32, tag="sx1")

        # dummy matmul reading only wmT: absorbs the ACT(copy)->PE dependency
        # so the j=0 matmul below carries a single DMA wait (walrus limit).
        dummy_ps = psacc.tile([1, 1], F32, tag="dummy")
        nc.tensor.matmul(dummy_ps[:], wmT[:, 0:1], wmT[:, 0:1])

        for rep, j in [(r, jj) for r in range(loop_mult) for jj in range(BL)]:
            xj = xpool.tile([S, V], F32, tag="xj")
            nc.sync.dma_start(xj[:], x_d[j])

            # running max along V; last column is the row max
            cm = cmpool.tile([S, V], F32, tag="cm")
            nc.vector.tensor_tensor_scan(
                cm[:], xj[:], xj[:], NEG_INF, OP.max, OP.bypass
            )
            # argmax = #positions strictly before the first max
            dcnt = dpool.tile([S, V], F16, tag="dumpc")
            nc.vector.tensor_scalar(
                dcnt[:], cm[:], cm[:, V - 1 : V], None, OP.is_lt, OP.add,
                accum_out=idxT[:, j : j + 1],
            )
            # sum of exp(x) along V (no max subtraction: |x|<6 is safe in fp32)
            de = dpool.tile([S, V], F32, tag="dumpe")
            nc.scalar.activation(
                de[:], xj[:], AF.Exp, accum_out=seT[:, j : j + 1]
            )
            # gather exp(x)[target] = one-hot(iota==t) * e, sum along V; the
            # [B,S] stage recovers x_t = ln(e_t). Reading e instead of x keeps
            # gpsimd off the x tile, so the x DMA has fewer WAR waits and the
            # gpsimd gather waits only on ACT.
            dg = dpool.tile([S, V], F16, tag="dumpg")
            on_gps = j < GATHER_ON_GPSIMD
            eng = nc.gpsimd if on_gps else nc.vector
            eng.scalar_tensor_tensor(
                dg[:], iota_v[:], tT[:, j : j + 1], de[:],
                OP.is_equal, OP.mult,
                accum_out=xtT[:, j : j + 1],
            )
            # sum_v x weighted by wm, accumulated over all rows in PSUM
            nc.tensor.matmul(
                sx0[:], wmT[:, j : j + 1], xj[:, 0:512],
                start=(rep == 0 and j == 0),
                stop=(rep == loop_mult - 1 and j == BL - 1),
            )
            nc.tensor.matmul(
                sx1[:], wmT[:, j : j + 1], xj[:, 512:V],
                start=(rep == 0 and j == 0),
                stop=(rep == loop_mult - 1 and j == BL - 1),
            )

        # ---- stage 3: [B,S] wrap-up ----
        lseT = cols.tile([S, BL], F32, tag="lseT")
        nc.scalar.activation(lseT[:], seT[:], AF.Ln)

        def transpose_back(src, tag):
            ps = psum.tile([BL, S], F32, tag="tpb")
            nc.tensor.transpose(ps[:], src[:], ident128[:])
            out = small.tile([BL, S], F32, tag=tag)
            nc.scalar.copy(out[:], ps[:])
            return out

        pred = transpose_back(idxT, "pred")
        # xtT columns are written by two engines (DVE + gpsimd); merge the
        # dependency through one DVE copy so downstream ops have single waits.
        xtT2 = cols.tile([S, BL], F32, tag="xtT2")
        nc.vector.tensor_copy(xtT2[:], xtT[:])
        # xtT holds e_t = exp(x_t); recover x_t
        xtlogT = cols.tile([S, BL], F32, tag="xtlogT")
        nc.scalar.activation(xtlogT[:], xtT2[:], AF.Ln)
        xt = transpose_back(xtlogT, "xt")
        lse = transpose_back(lseT, "lse")

        stats = small.tile([BL, 16], F32, tag="stats")
        nc.vector.memset(stats[:], 0.0)
        aux = small.tile([1, 4], F32, tag="aux")
        nc.vector.memset(aux[:], 0.0)

        dump_s = small.tile([BL, S], F32, tag="dumps")

        # c0: sum_s wm * (lse - 0.9*xt)
        ce1 = small.tile([BL, S], F32, tag="ce1")
        nc.vector.scalar_tensor_tensor(
            ce1[:], xt[:], -(1.0 - LABEL_SMOOTHING), lse[:], OP.mult, OP.add
        )
        nc.vector.tensor_tensor(dump_s[:], ce1[:], wm[:], OP.mult)
        nc.vector.tensor_reduce(stats[:, 0:1], dump_s[:], AX.X, OP.add)
        # c1: sum_s w
        nc.vector.tensor_reduce(stats[:, 1:2], w[:], AX.X, OP.add)
        # c2: |pred_len - L|
        prednz = small.tile([BL, S], F32, tag="prednz")
        nc.vector.tensor_scalar(prednz[:], pred[:], float(PAD_IDX), None, OP.not_equal)
        plen = small.tile([BL, 1], F32, tag="plen")
        nc.vector.tensor_reduce(plen[:], prednz[:], AX.X, OP.add)
        pdiff = small.tile([BL, 1], F32, tag="pdiff")
        nc.vector.tensor_tensor(pdiff[:], plen[:], L[:], OP.subtract)
        nc.scalar.activation(stats[:, 2:3], pdiff[:], AF.Abs)
        # c3/c4: char bigram/trigram squared sums
        pe = small.tile([BL, S - 1], F32, tag="pe")
        nc.vector.tensor_tensor(pe[:], pred[:, 0 : S - 1], pred[:, 1:S], OP.is_equal)
        te = small.tile([BL, S - 1], F32, tag="te")
        nc.vector.tensor_tensor(te[:], tgt_f[:, 0 : S - 1], tgt_f[:, 1:S], OP.is_equal)
        same = small.tile([BL, S - 1], F32, tag="same")
        nc.vector.tensor_tensor(
            same[:], pred[:, 0 : S - 1], tgt_f[:, 0 : S - 1], OP.is_equal
        )
        pt = small.tile([BL, S - 1], F32, tag="pt")
        nc.vector.tensor_tensor(pt[:], pe[:], te[:], OP.mult)
        spt = small.tile([BL, S - 1], F32, tag="spt")
        nc.vector.tensor_tensor(spt[:], pt[:], same[:], OP.mult)
        s1 = small.tile([BL, S - 1], F32, tag="s1")
        nc.vector.tensor_tensor(s1[:], pe[:], te[:], OP.add)
        bi = small.tile([BL, S - 1], F32, tag="bi")
        nc.vector.scalar_tensor_tensor(bi[:], spt[:], -2.0, s1[:], OP.mult, OP.add)
        nc.vector.tensor_reduce(stats[:, 3:4], bi[:], AX.X, OP.add)

        pe3 = small.tile([BL, S - 2], F32, tag="pe3")
        nc.vector.tensor_tensor(pe3[:], pe[:, 0 : S - 2], pe[:, 1 : S - 1], OP.mult)
        te3 = small.tile([BL, S - 2], F32, tag="te3")
        nc.vector.tensor_tensor(te3[:], te[:, 0 : S - 2], te[:, 1 : S - 1], OP.mult)
        pt3 = small.tile([BL, S - 2], F32, tag="pt3")
        nc.vector.tensor_tensor(pt3[:], pe3[:], te3[:], OP.mult)
        spt3 = small.tile([BL, S - 2], F32, tag="spt3")
        nc.vector.tensor_tensor(spt3[:], pt3[:], same[:, 0 : S - 2], OP.mult)
        s3 = small.tile([BL, S - 2], F32, tag="s3")
        nc.vector.tensor_tensor(s3[:], pe3[:], te3[:], OP.add)
        tri = small.tile([BL, S - 2], F32, tag="tri")
        nc.vector.scalar_tensor_tensor(tri[:], spt3[:], -2.0, s3[:], OP.mult, OP.add)
        nc.vector.tensor_reduce(stats[:, 4:5], tri[:], AX.X, OP.add)
        # c5: sum_s mask[:, :-2] (valid_tri partials)
        nc.vector.tensor_reduce(stats[:, 5:6], mask[:, 0 : S - 2], AX.X, OP.add)
        # c6: correct = (pred == target) & mask
        eqc = small.tile([BL, S], F32, tag="eqc")
        nc.vector.tensor_tensor(eqc[:], pred[:], tgt_f[:], OP.is_equal)
        dump_s2 = small.tile([BL, S], F32, tag="dumps2")
        nc.vector.tensor_tensor(dump_s2[:], eqc[:], mask[:], OP.mult)
        nc.vector.tensor_reduce(stats[:, 6:7], dump_s2[:], AX.X, OP.add)
        # c7: total chars per row (= L)
        nc.vector.tensor_copy(stats[:, 7:8], L[:])
        # c8: end char ok
        Lm1 = small.tile([BL, 1], F32, tag="Lm1")
        nc.vector.tensor_scalar(Lm1[:], L[:], 1.0, None, OP.subtract)
        eqL = small.tile([BL, S], F32, tag="eqL")
        nc.vector.tensor_scalar(eqL[:], iota_s[:], Lm1[:], None, OP.is_equal)
        dump_s3 = small.tile([BL, S], F32, tag="dumps3")
        nc.vector.tensor_tensor(dump_s3[:], eqL[:], eqc[:], OP.mult)
        nc.vector.tensor_reduce(stats[:, 8:9], dump_s3[:], AX.X, OP.add)
        # c9: length accuracy partials
        nc.vector.tensor_tensor(stats[:, 9:10], plen[:], L[:], OP.is_equal)

        # aux: the two PSUM halves of sum_pos wm * sum_v x
        nc.vector.tensor_reduce(aux[:, 0:1], sx0[:], AX.X, OP.add)
        nc.vector.tensor_reduce(aux[:, 1:2], sx1[:], AX.X, OP.add)

        nc.sync.dma_start(stats_d[:, :], stats[:])
        nc.sync.dma_start(aux_d[:, :], aux[:])

    nc.compile()
    return nc


_built = None


def _get_nc():
    global _built
    if _built is None:
        _built = build_bass()
    return _built


def combine(stats_list, aux_list):
    """Host-side psum of the per-core scalar partials + final formulas."""
    Ssum = np.zeros(16, dtype=np.float64)
    Asum = np.zeros(4, dtype=np.float64)
    for st in stats_list:
        Ssum += st.astype(np.float64).sum(axis=0)
    for ax in aux_list:
        Asum += ax.astype(np.float64).sum(axis=0)

    num = Ssum[0] - (LABEL_SMOOTHING / V) * (Asum[0] + Asum[1])
    den = Ssum[1]
    weighted_loss = num / den
    length_penalty = LENGTH_PENALTY * Ssum[2] / B
    bigram_mse = Ssum[3] / (B * (S - 1) * V)
    tri_mse = Ssum[4] / (B * (S - 2) * V)
    valid_tri = Ssum[5] > 0
    char_ngram = bigram_mse + (tri_mse if valid_tri else 0.0)
    total_loss = weighted_loss + length_penalty + CHAR_WEIGHT * char_ngram

    total_chars = Ssum[7]
    char_acc = Ssum[6] / total_chars if total_chars > 0 else 0.0
    end_char_acc = Ssum[8] / B
    length_acc = Ssum[9] / B
    f = np.float32
    return (f(total_loss), f(char_acc), f(end_char_acc), f(length_acc))


def kernel(output, target, _trace=False):
    output = np.ascontiguousarray(np.asarray(output, dtype=np.float32))
    target = np.ascontiguousarray(np.asarray(target, dtype=np.int32))
    nc = _get_nc()
    in_maps = [
        {
            "output": output[c * BL : (c + 1) * BL],
            "target": target[c * BL : (c + 1) * BL],
        }
        for c in range(NCORES)
    ]
    res = run_bass_kernel_spmd(nc, in_maps, list(range(NCORES)), trace=_trace)
    stats_list = [res.results[c]["stats"] for c in range(NCORES)]
    aux_list = [res.results[c]["aux"] for c in range(NCORES)]
    out = combine(stats_list, aux_list)
    if _trace:
        return out, res
    return out


# revision 24
# speedup vs baseline: 37.5135x; 1.5121x over previous
"""DAE loss kernel for Trainium2 (Bass/Tile), 8-core data parallel.

Shards the batch (512 -> 64 rows/core). Each core computes, per (b,s)
position: argmax over V (scan-cummax + count trick), x[target] (one-hot
scalar_tensor_tensor gather), log-sum-exp (ACT Exp with accumulate), and
a PE-matmul contraction for the label-smoothing mean term. A small [B,S]
stage reduces everything to 10 per-row partial sums; the host sums the
8x64 partial rows and applies the final scalar formulas.
"""

import numpy as np
from contextlib import ExitStack

import concourse.bass as bass
import concourse.bacc as bacc
import concourse.mybir as mybir
from concourse import tile
from concourse.bass_utils import run_bass_kernel_spmd


B, S, V = 512, 128, 1024
NCORES = 8
BL = B // NCORES  # 64 batch rows per core

PAD_IDX = 0
LABEL_SMOOTHING = 0.1
END_WEIGHT = 3.0
CHAR_WEIGHT = 0.2
LENGTH_PENALTY = 0.1

F32 = mybir.dt.float32
F16 = mybir.dt.float16
I32 = mybir.dt.int32
OP = mybir.AluOpType
AF = mybir.ActivationFunctionType
AX = mybir.AxisListType

NEG_INF = -3.0e38

# gpsimd cannot run scalar_tensor_tensor (walrus opcode-on-engine check),
# so the one-hot gather lives on the vector engine.
GATHER_ON_GPSIMD = 0


def build_bass(loop_mult=1):
    # loop_mult > 1 repeats the V-loop over the same data; used only by the
    # timing bench to separate device time from dispatch overhead.
    # Bacc (not raw Bass): its compile() splits multi-sem waits into
    # EventSemaphore chains — hardware allows one sync wait per instruction.
    nc = bacc.Bacc("TRN2", target_bir_lowering=False, debug=False, num_devices=NCORES)
    x_d = nc.dram_tensor("output", [BL, S, V], F32, kind="ExternalInput").ap()
    t_d = nc.dram_tensor("target", [BL, S], I32, kind="ExternalInput").ap()
    stats_d = nc.dram_tensor("stats", [BL, 16], F32, kind="ExternalOutput").ap()
    aux_d = nc.dram_tensor("aux", [1, 4], F32, kind="ExternalOutput").ap()

    with tile.TileContext(nc) as tc, ExitStack() as ctx:
        const = ctx.enter_context(tc.tile_pool(name="const", bufs=1))
        cols = ctx.enter_context(tc.tile_pool(name="cols", bufs=1))
        small = ctx.enter_context(tc.tile_pool(name="small", bufs=1))
        xpool = ctx.enter_context(tc.tile_pool(name="x", bufs=4))
        cmpool = ctx.enter_context(tc.tile_pool(name="cm", bufs=3))
        dpool = ctx.enter_context(tc.tile_pool(name="dump", bufs=6))
        psum = ctx.enter_context(tc.tile_pool(name="psum", bufs=2, space="PSUM"))
        psacc = ctx.enter_context(tc.tile_pool(name="psacc", bufs=1, space="PSUM"))

        # ---- constants ----
        iota_v_i = const.tile([128, V], I32, tag="iotavi")
        nc.gpsimd.iota(iota_v_i[:], [[1, V]], base=0, channel_multiplier=0)
        iota_v = const.tile([128, V], F32, tag="iotav")
        nc.vector.tensor_copy(iota_v[:], iota_v_i[:])

        # identities produced by DVE (not gpsimd) so each PE transpose
        # depends on a single engine: walrus allows ONE sync wait per Matmult.
        pcol_i = const.tile([128, 1], I32, tag="pcoli")
        nc.gpsimd.iota(pcol_i[:], [[0, 1]], base=0, channel_multiplier=1)
        pcolf = const.tile([128, 1], F32, tag="pcolf")
        nc.vector.tensor_copy(pcolf[:], pcol_i[:])
        ident128 = const.tile([128, 128], F32, tag="id128")
        nc.vector.tensor_scalar(
            ident128[:], iota_v[:, 0:128], pcolf[:], None, OP.is_equal
        )
        ident64 = const.tile([64, 64], F32, tag="id64")
        nc.vector.tensor_scalar(
            ident64[:], iota_v[0:64, 0:64], pcolf[0:64], None, OP.is_equal
        )

        iota_s_i = const.tile([BL, S], I32, tag="iotasi")
        nc.gpsimd.iota(iota_s_i[:], [[1, S]], base=0, channel_multiplier=0)
        iota_s = const.tile([BL, S], F32, tag="iotas")
        nc.vector.tensor_copy(iota_s[:], iota_s_i[:])

        # ---- stage 1: target-derived weights ----
        tgt_i = small.tile([BL, S], I32, tag="tgti")
        nc.sync.dma_start(tgt_i[:], t_d[:, :])
        tgt_f = small.tile([BL, S], F32, tag="tgtf")
        nc.vector.tensor_copy(tgt_f[:], tgt_i[:])

        mask = small.tile([BL, S], F32, tag="mask")
        nc.vector.tensor_scalar(mask[:], tgt_f[:], float(PAD_IDX), None, OP.not_equal)

        L = small.tile([BL, 1], F32, tag="L")
        nc.vector.tensor_reduce(L[:], mask[:], AX.X, OP.add)
        Lf = small.tile([BL, 1], F32, tag="Lf")
        nc.vector.tensor_scalar(Lf[:], L[:], 1.0, None, OP.max)
        rec = small.tile([BL, 1], F32, tag="rec")
        nc.vector.reciprocal(rec[:], Lf[:])
        halfinv = small.tile([BL, 1], F32, tag="halfinv")
        nc.vector.tensor_scalar(halfinv[:], rec[:], 0.5, None, OP.mult)

        base = small.tile([BL, S], F32, tag="base")
        nc.vector.tensor_scalar(base[:], iota_s[:], halfinv[:], None, OP.mult)
        cmplt = small.tile([BL, S], F32, tag="cmplt")
        nc.vector.tensor_scalar(cmplt[:], iota_s[:], L[:], None, OP.is_lt)
        w = small.tile([BL, S], F32, tag="w")
        nc.vector.tensor_tensor(w[:], base[:], cmplt[:], OP.mult)
        nc.vector.tensor_scalar(w[:], w[:], 1.0, None, OP.add)

        # end-of-sequence boosts; positions L-3, L-2, L-1 are distinct so the
        # replacements commute (negative L-k never matches iota_s >= 0).
        for k, c in ((3, END_WEIGHT * 0.6), (2, END_WEIGHT * 0.8), (1, END_WEIGHT)):
            Lk = small.tile([BL, 1], F32, tag=f"Lk{k}")
            nc.vector.tensor_scalar(Lk[:], L[:], float(k), None, OP.subtract)
            eqk = small.tile([BL, S], F32, tag=f"eqk{k}")
            nc.vector.tensor_scalar(eqk[:], iota_s[:], Lk[:], None, OP.is_equal)
            d = small.tile([BL, S], F32, tag=f"d{k}")
            nc.vector.tensor_scalar(d[:], w[:], -1.0, float(c), OP.mult, OP.add)
            nc.vector.tensor_tensor(d[:], d[:], eqk[:], OP.mult)
            nc.vector.tensor_tensor(w[:], w[:], d[:], OP.add)

        wm = small.tile([BL, S], F32, tag="wm")
        nc.vector.tensor_tensor(wm[:], w[:], mask[:], OP.mult)

        # transposes to [128(s), 64(b)] for per-tile per-partition scalars
        wmT_ps = psum.tile([S, BL], F32, tag="tps")
        nc.tensor.transpose(wmT_ps[:], wm[:], ident64[:])
        wmT = cols.tile([S, BL], F32, tag="wmT")
        nc.scalar.copy(wmT[:], wmT_ps[:])

        tT_ps = psum.tile([S, BL], F32, tag="tps")
        nc.tensor.transpose(tT_ps[:], tgt_f[:], ident64[:])
        tT = cols.tile([S, BL], F32, tag="tT")
        nc.scalar.copy(tT[:], tT_ps[:])

        # gpsimd allows only one sync wait per instruction, and its gathers
        # must wait on their x-tile DMA. Absorb the iota_v/tT producer waits
        # into tiny same-engine probe copies ahead of the loop so the gathers'
        # vector clock is already up to date for those operands.
        probe = const.tile([1, 2], F32, tag="probe")
        nc.gpsimd.tensor_copy(probe[:, 0:1], iota_v[0:1, 0:1])
        nc.gpsimd.tensor_copy(probe[:, 1:2], tT[0:1, 0:1])

        # ---- stage 2: the V loop ----
        idxT = cols.tile([S, BL], F32, tag="idxT")
        xtT = cols.tile([S, BL], F32, tag="xtT")
        seT = cols.tile([S, BL], F32, tag="seT")

        sx0 = psacc.tile([1, 512], F32, tag="sx0")
        sx1 = psacc.tile([1, 512], F32, tag="sx1")

        # dummy matmul reading only wmT: absorbs the ACT(copy)->PE dependency
        # so the j=0 matmul below carries a single DMA wait (walrus limit).
        dummy_ps = psacc.tile([1, 1], F32, tag="dummy")
        nc.tensor.matmul(dummy_ps[:], wmT[:, 0:1], wmT[:, 0:1])

        for rep, j in [(r, jj) for r in range(loop_mult) for jj in range(BL)]:
            xj = xpool.tile([S, V], F32, tag="xj")
            nc.sync.dma_start(xj[:], x_d[j])

            # running max along V; last column is the row max
            cm = cmpool.tile([S, V], F32, tag="cm")
            nc.vector.tensor_tensor_scan(
                cm[:], xj[:], xj[:], NEG_INF, OP.max, OP.bypass
            )
            # argmax = #positions strictly before the first max, counted on
            # ACT: sign(m - cm) is +1 before the first max and 0 from it on,
            # so the activation accumulator yields the index directly.
            dcnt = dpool.tile([S, V], F16, tag="dumpc")
            nc.scalar.activation(
                dcnt[:], cm[:], AF.Sign, bias=cm[:, V - 1 : V], scale=-1.0,
                accum_out=idxT[:, j : j + 1],
            )
            # sum of exp(x) along V (no max subtraction: |x|<6 is safe in fp32)
            de = dpool.tile([S, V], F32, tag="dumpe")
            nc.scalar.activation(
                de[:], xj[:], AF.Exp, accum_out=seT[:, j : j + 1]
            )
            # gather exp(x)[target] = one-hot(iota==t) * e, sum along V; the
            # [B,S] stage recovers x_t = ln(e_t). Reading e instead of x keeps
            # gpsimd off the x tile, so the x DMA has fewer WAR waits and the
            # gpsimd gather waits only on ACT.
            dg = dpool.tile([S, V], F16, tag="dumpg")
            on_gps = j < GATHER_ON_GPSIMD
            eng = nc.gpsimd if on_gps else nc.vector
            eng.scalar_tensor_tensor(
                dg[:], iota_v[:], tT[:, j : j + 1], de[:],
                OP.is_equal, OP.mult,
                accum_out=xtT[:, j : j + 1],
            )
            # sum_v x weighted by wm, accumulated over all rows in PSUM
            nc.tensor.matmul(
                sx0[:], wmT[:, j : j + 1], xj[:, 0:512],
                start=(rep == 0 and j == 0),
                stop=(rep == loop_mult - 1 and j == BL - 1),
            )
            nc.tensor.matmul(
                sx1[:], wmT[:, j : j + 1], xj[:, 512:V],
                start=(rep == 0 and j == 0),
                stop=(rep == loop_mult - 1 and j == BL - 1),
            )

        # ---- stage 3: [B,S] wrap-up ----
        lseT = cols.tile([S, BL], F32, tag="lseT")
        nc.scalar.activation(lseT[:], seT[:], AF.Ln)

        def transpose_back(src, tag):
            ps = psum.tile([BL, S], F32, tag="tpb")
            nc.tensor.transpose(ps[:], src[:], ident128[:])
            out = small.tile([BL, S], F32, tag=tag)
            nc.scalar.copy(out[:], ps[:])
            return out

        pred = transpose_back(idxT, "pred")
        # xtT columns are written by two engines (DVE + gpsimd); merge the
        # dependency through one DVE copy so downstream ops have single waits.
        xtT2 = cols.tile([S, BL], F32, tag="xtT2")
        nc.vector.tensor_copy(xtT2[:], xtT[:])
        # xtT holds e_t = exp(x_t); recover x_t
        xtlogT = cols.tile([S, BL], F32, tag="xtlogT")
        nc.scalar.activation(xtlogT[:], xtT2[:], AF.Ln)
        xt = transpose_back(xtlogT, "xt")
        lse = transpose_back(lseT, "lse")

        stats = small.tile([BL, 16], F32, tag="stats")
        nc.vector.memset(stats[:], 0.0)
        aux = small.tile([1, 4], F32, tag="aux")
        nc.vector.memset(aux[:], 0.0)

        dump_s = small.tile([BL, S], F32, tag="dumps")

        # c0: sum_s wm * (lse - 0.9*xt)
        ce1 = small.tile([BL, S], F32, tag="ce1")
        nc.vector.scalar_tensor_tensor(
            ce1[:], xt[:], -(1.0 - LABEL_SMOOTHING), lse[:], OP.mult, OP.add
        )
        nc.vector.tensor_tensor(dump_s[:], ce1[:], wm[:], OP.mult)
        nc.vector.tensor_reduce(stats[:, 0:1], dump_s[:], AX.X, OP.add)
        # c1: sum_s w
        nc.vector.tensor_reduce(stats[:, 1:2], w[:], AX.X, OP.add)
        # c2: |pred_len - L|
        prednz = small.tile([BL, S], F32, tag="prednz")
        nc.vector.tensor_scalar(prednz[:], pred[:], float(PAD_IDX), None, OP.not_equal)
        plen = small.tile([BL, 1], F32, tag="plen")
        nc.vector.tensor_reduce(plen[:], prednz[:], AX.X, OP.add)
        pdiff = small.tile([BL, 1], F32, tag="pdiff")
        nc.vector.tensor_tensor(pdiff[:], plen[:], L[:], OP.subtract)
        nc.scalar.activation(stats[:, 2:3], pdiff[:], AF.Abs)
        # c3/c4: char bigram/trigram squared sums
        pe = small.tile([BL, S - 1], F32, tag="pe")
        nc.vector.tensor_tensor(pe[:], pred[:, 0 : S - 1], pred[:, 1:S], OP.is_equal)
        te = small.tile([BL, S - 1], F32, tag="te")
        nc.vector.tensor_tensor(te[:], tgt_f[:, 0 : S - 1], tgt_f[:, 1:S], OP.is_equal)
        same = small.tile([BL, S - 1], F32, tag="same")
        nc.vector.tensor_tensor(
            same[:], pred[:, 0 : S - 1], tgt_f[:, 0 : S - 1], OP.is_equal
        )
        pt = small.tile([BL, S - 1], F32, tag="pt")
        nc.vector.tensor_tensor(pt[:], pe[:], te[:], OP.mult)
        spt = small.tile([BL, S - 1], F32, tag="spt")
        nc.vector.tensor_tensor(spt[:], pt[:], same[:], OP.mult)
        s1 = small.tile([BL, S - 1], F32, tag="s1")
        nc.vector.tensor_tensor(s1[:], pe[:], te[:], OP.add)
        bi = small.tile([BL, S - 1], F32, tag="bi")
        nc.vector.scalar_tensor_tensor(bi[:], spt[:], -2.0, s1[:], OP.mult, OP.add)
        nc.vector.tensor_reduce(stats[:, 3:4], bi[:], AX.X, OP.add)

        pe3 = small.tile([BL, S - 2], F32, tag="pe3")
        nc.vector.tensor_tensor(pe3[:], pe[:, 0 : S - 2], pe[:, 1 : S - 1], OP.mult)
        te3 = small.tile([BL, S - 2], F32, tag="te3")
        nc.vector.tensor_tensor(te3[:], te[:, 0 : S - 2], te[:, 1 : S - 1], OP.mult)
        pt3 = small.tile([BL, S - 2], F32, tag="pt3")
        nc.vector.tensor_tensor(pt3[:], pe3[:], te3[:], OP.mult)
        spt3 = small.tile([BL, S - 2], F32, tag="spt3")
        nc.vector.tensor_tensor(spt3[:], pt3[:], same[:, 0 : S - 2], OP.mult)
        s3 = small.tile([BL, S - 2], F32, tag="s3")
        nc.vector.tensor_tensor(s3[:], pe3[:], te3[:], OP.add)
        tri = small.tile([BL, S - 2], F32, tag="tri")
        nc.vector.scalar_tensor_tensor(tri[:], spt3[:], -2.0, s3[:], OP.mult, OP.add)
        nc.vector.tensor_reduce(stats[:, 4:5], tri[:], AX.X, OP.add)
        # c5: sum_s mask[:, :-2] (valid_tri partials)
        nc.vector.tensor_reduce(stats[:, 5:6], mask[:, 0 : S - 2], AX.X, OP.add)
        # c6: correct = (pred == target) & mask
        eqc = small.tile([BL, S], F32, tag="eqc")
        nc.vector.tensor_tensor(eqc[:], pred[:], tgt_f[:], OP.is_equal)
        dump_s2 = small.tile([BL, S], F32, tag="dumps2")
        nc.vector.tensor_tensor(dump_s2[:], eqc[:], mask[:], OP.mult)
        nc.vector.tensor_reduce(stats[:, 6:7], dump_s2[:], AX.X, OP.add)
        # c7: total chars per row (= L)
        nc.vector.tensor_copy(stats[:, 7:8], L[:])
        # c8: end char ok
        Lm1 = small.tile([BL, 1], F32, tag="Lm1")
        nc.vector.tensor_scalar(Lm1[:], L[:], 1.0, None, OP.subtract)
        eqL = small.tile([BL, S], F32, tag="eqL")
        nc.vector.tensor_scalar(eqL[:], iota_s[:], Lm1[:], None, OP.is_equal)
        dump_s3 = small.tile([BL, S], F32, tag="dumps3")
        nc.vector.tensor_tensor(dump_s3[:], eqL[:], eqc[:], OP.mult)
        nc.vector.tensor_reduce(stats[:, 8:9], dump_s3[:], AX.X, OP.add)
        # c9: length accuracy partials
        nc.vector.tensor_tensor(stats[:, 9:10], plen[:], L[:], OP.is_equal)

        # aux: the two PSUM halves of sum_pos wm * sum_v x
        nc.vector.tensor_reduce(aux[:, 0:1], sx0[:], AX.X, OP.add)
        nc.vector.tensor_reduce(aux[:, 1:2], sx1[:], AX.X, OP.add)

        nc.sync.dma_start(stats_d[:, :], stats[:])
        nc.sync.dma_start(aux_d[:, :], aux[:])

    nc.compile()
    return nc


_built = None


def _get_nc():
    global _built
    if _built is None:
        _built = build_bass()
    return _built


def combine(stats_list, aux_list):
    """Host-side psum of the per-core scalar partials + final formulas."""
    Ssum = np.zeros(16, dtype=np.float64)
    Asum = np.zeros(4, dtype=np.float64)
    for st in stats_list:
        Ssum += st.astype(np.float64).sum(axis=0)
    for ax in aux_list:
        Asum += ax.astype(np.float64).sum(axis=0)

    num = Ssum[0] - (LABEL_SMOOTHING / V) * (Asum[0] + Asum[1])
    den = Ssum[1]
    weighted_loss = num / den
    length_penalty = LENGTH_PENALTY * Ssum[2] / B
    bigram_mse = Ssum[3] / (B * (S - 1) * V)
    tri_mse = Ssum[4] / (B * (S - 2) * V)
    valid_tri = Ssum[5] > 0
    char_ngram = bigram_mse + (tri_mse if valid_tri else 0.0)
    total_loss = weighted_loss + length_penalty + CHAR_WEIGHT * char_ngram

    total_chars = Ssum[7]
    char_acc = Ssum[6] / total_chars if total_chars > 0 else 0.0
    end_char_acc = Ssum[8] / B
    length_acc = Ssum[9] / B
    f = np.float32
    return (f(total_loss), f(char_acc), f(end_char_acc), f(length_acc))


def kernel(output, target, _trace=False):
    output = np.ascontiguousarray(np.asarray(output, dtype=np.float32))
    target = np.ascontiguousarray(np.asarray(target, dtype=np.int32))
    nc = _get_nc()
    in_maps = [
        {
            "output": output[c * BL : (c + 1) * BL],
            "target": target[c * BL : (c + 1) * BL],
        }
        for c in range(NCORES)
    ]
    res = run_bass_kernel_spmd(nc, in_maps, list(range(NCORES)), trace=_trace)
    stats_list = [res.results[c]["stats"] for c in range(NCORES)]
    aux_list = [res.results[c]["aux"] for c in range(NCORES)]
    out = combine(stats_list, aux_list)
    if _trace:
        return out, res
    return out
